# revision 14
# baseline (speedup 1.0000x reference)
"""Fused int8 dequant -> causal mask -> softmax -> int8 requant on 8 TRN2 cores.

Problem: x_q [B=4, H=16, S=1024, S] int8, per-(head,row) scales sx/so [H*S] f32.
  out = int8(clip(round(softmax(causal_mask(x_q * sx)) / so), -128, 127))

Sharding: 2 heads per core (data parallel over 64 independent (b, h) planes).
Rows live on partitions; softmax runs along the free dim. For each (h, t)
row-tile of 128 rows, only cols [0, W=(t+1)*128) can be nonzero (causal), so
only those are moved. Host-side prep packs x premasked (strict upper triangle
zeroed) into per-(h,t) blocks [128, B*W]; host-side unpack re-applies the
tril mask on the diagonal 128x128 block of each row-tile (so the device never
spends time zeroing masked lanes).

Engine budget (measured rates: ACT instr 369ns + 0.833ns/elem/lane, accum
readout 280ns; DVE fp16 tensor_scalar 2x = 0.52ns/elem, tensor_tensor 2x,
reductions 1x = 1.04ns/elem, small-op ~150-220ns/instr):

  ACT: one batched Exp per (h, t<=4) block [P, B*W] (row sums for these
       tiles are cheaper on DVE), per-b Exp+accum_out for t>=5 (large tiles,
       where DVE's 1x reduce tax exceeds ACT's per-instr+readout tax).
  DVE: row sums for t<=4 via a log-tree of fp16 tensor_tensor halving adds
       (2x mode) followed by one 1x tensor_reduce on the shrunk tile;
       r = 1/((sum - corr)*so) via one fused scalar_tensor_tensor +
       reciprocal; requant y = et * r -> int8 per (t,b) (2x mode).
  Premasked x makes masked lanes contribute exp(0)=1, corrected by the
  compile-time constant (127 - p) before use.

(fp16 et: element rounding gives end-to-end absmax diff 1 vs the f32
reference; sums accumulate in f32, halving partials stay in fp16.)
"""

import contextlib
import ctypes
import os
import sys
import types
from contextlib import ExitStack

import numpy as np

import concourse.bacc as bacc
import concourse.bass as bass
import concourse.tile as tile
from concourse import mybir
from concourse.bass_utils import run_bass_kernel_spmd

B, H, S = 4, 16, 1024
NCORES = 8
HPC = H // NCORES  # heads per core
P = 128
NT = S // P  # row tiles per plane
AF = mybir.ActivationFunctionType
ALU = mybir.AluOpType
AX = mybir.AxisListType

# packed block offsets: block (h, t) holds [P, B*W] int8, W = (t+1)*P
_BLK = [[None] * NT for _ in range(HPC)]
_off = 0
for _h in range(HPC):
    for _t in range(NT):
        _W = (_t + 1) * P
        _BLK[_h][_t] = (_off, _W)
        _off += P * B * _W
TOTAL = _off  # per-core packed bytes (4718592)

# tiles t < ACT_SUM_T0 sum on DVE (halving tree + tensor_reduce); the rest
# use per-b exp+accum on ACT. LEVELS[t] = halving-tree depth for DVE tiles.
ACT_SUM_T0 = 5
LEVELS = {0: 0, 1: 1, 2: 2, 3: 2, 4: 3}

_AXON_SO = "/opt/axon/libaxon_pjrt.so"


def _ensure_ntff_hook():
    """This image's antenv lacks axon_hooks; provide it so trace=True works."""
    if "antenv.axon_hooks" in sys.modules:
        return
    import antenv

    mod = types.ModuleType("antenv.axon_hooks")
    state = {"hook": None}
    mod.set_axon_ntff_profile_hook = lambda h: state.__setitem__("hook", h)
    mod.get_axon_ntff_profile_hook = lambda: state["hook"]
    sys.modules["antenv.axon_hooks"] = mod
    antenv.axon_hooks = mod

    if not os.path.exists(_AXON_SO):
        return
    lib = ctypes.CDLL(_AXON_SO)
    if not hasattr(lib, "axon_start_nrt_profile"):
        return
    lib.axon_start_nrt_profile.argtypes = [ctypes.POINTER(ctypes.c_int64), ctypes.c_size_t]
    lib.axon_start_nrt_profile.restype = ctypes.c_int64
    lib.axon_stop_nrt_profile.argtypes = [ctypes.c_char_p]
    lib.axon_stop_nrt_profile.restype = ctypes.c_int64

    @contextlib.contextmanager
    def _hook(output_dir, device_ids):
        import jax

        jax.devices()
        if device_ids:
            ids = (ctypes.c_int64 * len(device_ids))(*device_ids)
            rc = lib.axon_start_nrt_profile(ids, len(device_ids))
        else:
            rc = lib.axon_start_nrt_profile(None, 0)
        if rc != 0:
            raise RuntimeError(f"axon_start_nrt_profile rc={rc}")
        try:
            yield
        finally:
            n = lib.axon_stop_nrt_profile(str(output_dir).encode())
            print(f"profile: {n} file(s) written to {output_dir}", file=sys.stderr)

    mod.set_axon_ntff_profile_hook(_hook)


_cached_nc = None


def _ap3(t, off_elems, pdim, d1, d2):
    """3D AP view [partitions, d1, d2] of tile t at element offset off_elems."""
    return bass.AP(tensor=t.tensor, offset=t.offset + off_elems,
                   ap=[t.ap[0], d1, d2])


def _build_bass(compile=True):
    nc = bacc.Bacc("TRN2", target_bir_lowering=False, debug=False,
                   num_devices=NCORES)
    x = nc.declare_dram_parameter("x", [TOTAL], mybir.dt.int8, isOutput=False)
    # sc packs sx | so | corr as one [P, 2*HPC*NT + 1] f32 block (one DMA)
    NCOL = HPC * NT
    sc = nc.declare_dram_parameter("sc", [P, 2 * NCOL + 1], mybir.dt.float32,
                                   isOutput=False)
    y = nc.declare_dram_parameter("y", [TOTAL], mybir.dt.int8, isOutput=True)

    with ExitStack() as ctx:
        tc = ctx.enter_context(tile.TileContext(nc))
        singles = ctx.enter_context(tc.tile_pool(name="singles", bufs=1))
        xpool = ctx.enter_context(tc.tile_pool(name="xp", bufs=7))
        epool = ctx.enter_context(tc.tile_pool(name="ep", bufs=5))
        spool = ctx.enter_context(tc.tile_pool(name="sp", bufs=4))
        ypool = ctx.enter_context(tc.tile_pool(name="yp", bufs=7))
        smalls = ctx.enter_context(tc.tile_pool(name="sm", bufs=8))

        # dummy activation on scratch SBUF: walrus emits the exp table load
        # right before it, so the ~1.3us ACT_TABLE_LOAD overlaps the first
        # DMA wait instead of serializing before the first real exp
        dummy = singles.tile([P, 1], mybir.dt.float32)
        nc.scalar.activation(dummy[:], dummy[:], AF.Exp, bias=0.0, scale=0.0)

        sct = singles.tile([P, 2 * NCOL + 1], mybir.dt.float32)
        nc.sync.dma_start(sct[:], sc[:])
        corrt = sct[:, 2 * NCOL:2 * NCOL + 1]

        def emit_exp_and_sums(h, t, sums, base):
            """DMA x in, exp -> et, row sums -> sums[:, base:base+B]."""
            off, W = _BLK[h][t]
            col = h * NT + t
            xt = xpool.tile([P, B * W], mybir.dt.int8, tag="xt")
            nc.sync.dma_start(
                xt[:], x[off:off + P * B * W].rearrange("(p n) -> p n", p=P))
            et = epool.tile([P, B * W], mybir.dt.float16, tag="et")
            if t < ACT_SUM_T0:
                # batched exp; row sums on DVE: halve in 2x fp16 adds,
                # then one 1x tensor_reduce over the shrunk block
                nc.scalar.activation(et[:], xt[:], AF.Exp, bias=0.0,
                                     scale=sct[:, col:col + 1])
                cur, w = et, W
                for lev in range(LEVELS[t]):
                    w2 = w // 2
                    # fixed max-size scratch per level (pool tags want
                    # stable shapes); APs below use only B*w2 elements
                    scr = spool.tile([P, 1280 >> lev],
                                     mybir.dt.float16, tag=f"scr{lev}")
                    nc.vector.tensor_tensor(
                        _ap3(scr, 0, P, [w2, B], [1, w2]),
                        _ap3(cur, 0, P, [w, B], [1, w2]),
                        _ap3(cur, w2, P, [w, B], [1, w2]),
                        ALU.add)
                    cur, w = scr, w2
                nc.vector.tensor_reduce(
                    sums[:, base:base + B], _ap3(cur, 0, P, [w, B], [1, w]),
                    AX.X, ALU.add)
            else:
                # per-b exp with free row sums from the ACT accumulator
                for b in range(B):
                    nc.scalar.activation(et[:, b * W:(b + 1) * W],
                                         xt[:, b * W:(b + 1) * W],
                                         AF.Exp, bias=0.0,
                                         scale=sct[:, col:col + 1],
                                         accum_out=sums[:, base + b:base + b + 1])
            return et

        def emit_requant(h, t, et, rt, base):
            """yt = et * rt[:, base+b] -> int8, DMA out."""
            off, W = _BLK[h][t]
            yt = ypool.tile([P, B * W], mybir.dt.int8, tag="yt")
            for b in range(B):
                nc.vector.tensor_scalar(yt[:, b * W:(b + 1) * W],
                                        et[:, b * W:(b + 1) * W],
                                        rt[:, base + b:base + b + 1], None,
                                        ALU.mult)
            nc.sync.dma_start(
                y[off:off + P * B * W].rearrange("(p n) -> p n", p=P), yt[:])

        # ascending order keeps ACT gapless (small DMAs land first, engines
        # self-balance); h1 rotates so the final tile is t0, whose short
        # post-exp chain (reduce+smalls+requant+64KB DMA) minimizes the tail.
        # Mid-stream adjacent pairs share one fused r = 1/((sum-corr)*so)
        # chain (halves the DVE small-op count there); endpoints stay
        # unpaired so the pipeline start/tail aren't serialized on a pair.
        GRPS0 = [[0], [1], [2, 3], [4], [5], [6, 7]]
        GRPS1 = [[1], [2, 3], [4], [6, 7], [5], [0]]
        for h in range(HPC):
            for grp in (GRPS0 if h == 0 else GRPS1):
                n = len(grp)
                sums = smalls.tile([P, 2 * B], mybir.dt.float32, tag="sums")
                ets = {}
                for i, t in enumerate(grp):
                    ets[t] = emit_exp_and_sums(h, t, sums, i * B)
                # r = 1/((sum - corr) * so) for the group at once; grp is
                # ascending so cols are adjacent and one strided AP covers it
                rt = smalls.tile([P, 2 * B], mybir.dt.float32, tag="rt")
                col0 = h * NT + grp[0]
                so_b = bass.AP(tensor=sct.tensor,
                               offset=sct.offset + NCOL + col0,
                               ap=[sct.ap[0], [1, n], [0, B]])
                nc.vector.scalar_tensor_tensor(
                    _ap3(rt, 0, P, [B, n], [1, B]),
                    _ap3(sums, 0, P, [B, n], [1, B]),
                    corrt, so_b, ALU.subtract, ALU.mult)
                nc.vector.reciprocal_approx_fast(rt[:, 0:n * B], rt[:, 0:n * B])
                for i, t in enumerate(grp):
                    emit_requant(h, t, ets[t], rt, i * B)
    if compile:
        nc.compile()
    return nc


_tril_mask = None
_tril_small = None


def _host_prep(x_q, scale_x, scale_out):
    global _tril_mask
    x_q = np.asarray(x_q)
    assert x_q.dtype == np.int8, x_q.dtype
    scale_x = np.asarray(scale_x, dtype=np.float32).reshape(H, S)
    scale_out = np.asarray(scale_out, dtype=np.float32).reshape(H, S)

    if _tril_mask is None:
        _tril_mask = np.tril(np.ones((S, S), dtype=np.int8))
    x_pm = x_q * _tril_mask  # zero the strict upper triangle

    # [P, H, NT]: sxr[p, h, t] = scale_x[h, t*128 + p]
    sxr = scale_x.reshape(H, NT, P).transpose(2, 0, 1)
    sor = scale_out.reshape(H, NT, P).transpose(2, 0, 1)

    corr = (127 - np.arange(P)).astype(np.float32).reshape(P, 1)

    NCOL = HPC * NT
    in_maps = []
    for c in range(NCORES):
        xc = np.empty(TOTAL, np.int8)
        for h in range(HPC):
            hg = c * HPC + h
            for t in range(NT):
                off, W = _BLK[h][t]
                # [B, P, W] -> [P, B, W] flattened
                blk = x_pm[:, hg, t * P:(t + 1) * P, 0:W].transpose(1, 0, 2)
                xc[off:off + P * B * W] = blk.reshape(-1)
        hs = slice(c * HPC, (c + 1) * HPC)
        scc = np.empty((P, 2 * NCOL + 1), np.float32)
        scc[:, 0:NCOL] = sxr[:, hs].reshape(P, NCOL)
        scc[:, NCOL:2 * NCOL] = sor[:, hs].reshape(P, NCOL)
        scc[:, 2 * NCOL:] = corr
        in_maps.append({"x": xc, "sc": scc})
    return in_maps


def _host_unpack(results):
    global _tril_small
    if _tril_small is None:
        _tril_small = np.tril(np.ones((P, P), dtype=np.int8))
    out = np.zeros((B, H, S, S), np.int8)
    for c in range(NCORES):
        yc = np.asarray(results[c]["y"])
        for h in range(HPC):
            hg = c * HPC + h
            for t in range(NT):
                off, W = _BLK[h][t]
                blk = yc[off:off + P * B * W].reshape(P, B, W).transpose(1, 0, 2)
                out[:, hg, t * P:(t + 1) * P, 0:W] = blk
                # mask the strict upper triangle of the diagonal block
                out[:, hg, t * P:(t + 1) * P, t * P:(t + 1) * P] *= _tril_small[None]
    return out


def run(x_q, scale_x, scale_out, trace=False):
    global _cached_nc
    if trace:
        _ensure_ntff_hook()
    if _cached_nc is None:
        _cached_nc = _build_bass()
    in_maps = _host_prep(x_q, scale_x, scale_out)
    res = run_bass_kernel_spmd(_cached_nc, in_maps, core_ids=list(range(NCORES)),
                               trace=trace)
    return _host_unpack(res.results), res


def kernel(x_q, scale_x, scale_out):
    out, _ = run(x_q, scale_x, scale_out,
                 trace=bool(int(os.environ.get("KERNEL_TRACE", "0"))))
    return out


# revision 18
# speedup vs baseline: 1.0123x; 1.0123x over previous
"""Fused int8 dequant -> causal mask -> softmax -> int8 requant on 8 TRN2 cores.

Problem: x_q [B=4, H=16, S=1024, S] int8, per-(head,row) scales sx/so [H*S] f32.
  out = int8(clip(round(softmax(causal_mask(x_q * sx)) / so), -128, 127))

Sharding: 2 heads per core (data parallel over 64 independent (b, h) planes).
Rows live on partitions; softmax runs along the free dim. For each (h, t)
row-tile of 128 rows, only cols [0, W=(t+1)*128) can be nonzero (causal), so
only those are moved. Host-side prep packs x premasked (strict upper triangle
zeroed) into per-(h,t) blocks [128, B*W]; host-side unpack re-applies the
tril mask on the diagonal 128x128 block of each row-tile (so the device never
spends time zeroing masked lanes).

Engine budget (measured rates: ACT instr 369ns + 0.833ns/elem/lane, accum
readout 280ns; DVE fp16 tensor_scalar 2x = 0.52ns/elem, tensor_tensor 2x,
reductions 1x = 1.04ns/elem, small-op ~150-220ns/instr):

  ACT: one batched Exp per (h, t<=4) block [P, B*W] (row sums for these
       tiles are cheaper on DVE), per-b Exp+accum_out for t>=5 (large tiles,
       where DVE's 1x reduce tax exceeds ACT's per-instr+readout tax).
  DVE: row sums for t<=4 via a log-tree of fp16 tensor_tensor halving adds
       (2x mode) followed by one 1x tensor_reduce on the shrunk tile;
       r = 1/((sum - corr)*so) via one fused scalar_tensor_tensor +
       reciprocal; requant y = et * r -> int8 per (t,b) (2x mode).
  Premasked x makes masked lanes contribute exp(0)=1, corrected by the
  compile-time constant (127 - p) before use.

(fp16 et: element rounding gives end-to-end absmax diff 1 vs the f32
reference; sums accumulate in f32, halving partials stay in fp16.)
"""

import contextlib
import ctypes
import os
import sys
import types
from contextlib import ExitStack

import numpy as np

import concourse.bacc as bacc
import concourse.bass as bass
import concourse.tile as tile
from concourse import mybir
from concourse.bass_utils import run_bass_kernel_spmd

B, H, S = 4, 16, 1024
NCORES = 8
HPC = H // NCORES  # heads per core
P = 128
NT = S // P  # row tiles per plane
AF = mybir.ActivationFunctionType
ALU = mybir.AluOpType
AX = mybir.AxisListType

# packed block offsets: block (h, t) holds [P, B*W] int8, W = (t+1)*P
_BLK = [[None] * NT for _ in range(HPC)]
_off = 0
for _h in range(HPC):
    for _t in range(NT):
        _W = (_t + 1) * P
        _BLK[_h][_t] = (_off, _W)
        _off += P * B * _W
TOTAL = _off  # per-core packed bytes (4718592)

# tiles t < ACT_SUM_T0 sum on DVE (halving tree + tensor_reduce); the rest
# use per-b exp+accum on ACT. LEVELS[t] = halving-tree depth for DVE tiles.
ACT_SUM_T0 = 5
LEVELS = {0: 0, 1: 1, 2: 2, 3: 2, 4: 3}

_AXON_SO = "/opt/axon/libaxon_pjrt.so"


def _ensure_ntff_hook():
    """This image's antenv lacks axon_hooks; provide it so trace=True works."""
    if "antenv.axon_hooks" in sys.modules:
        return
    import antenv

    mod = types.ModuleType("antenv.axon_hooks")
    state = {"hook": None}
    mod.set_axon_ntff_profile_hook = lambda h: state.__setitem__("hook", h)
    mod.get_axon_ntff_profile_hook = lambda: state["hook"]
    sys.modules["antenv.axon_hooks"] = mod
    antenv.axon_hooks = mod

    if not os.path.exists(_AXON_SO):
        return
    lib = ctypes.CDLL(_AXON_SO)
    if not hasattr(lib, "axon_start_nrt_profile"):
        return
    lib.axon_start_nrt_profile.argtypes = [ctypes.POINTER(ctypes.c_int64), ctypes.c_size_t]
    lib.axon_start_nrt_profile.restype = ctypes.c_int64
    lib.axon_stop_nrt_profile.argtypes = [ctypes.c_char_p]
    lib.axon_stop_nrt_profile.restype = ctypes.c_int64

    @contextlib.contextmanager
    def _hook(output_dir, device_ids):
        import jax

        jax.devices()
        if device_ids:
            ids = (ctypes.c_int64 * len(device_ids))(*device_ids)
            rc = lib.axon_start_nrt_profile(ids, len(device_ids))
        else:
            rc = lib.axon_start_nrt_profile(None, 0)
        if rc != 0:
            raise RuntimeError(f"axon_start_nrt_profile rc={rc}")
        try:
            yield
        finally:
            n = lib.axon_stop_nrt_profile(str(output_dir).encode())
            print(f"profile: {n} file(s) written to {output_dir}", file=sys.stderr)

    mod.set_axon_ntff_profile_hook(_hook)


_cached_nc = None


def _ap3(t, off_elems, pdim, d1, d2):
    """3D AP view [partitions, d1, d2] of tile t at element offset off_elems."""
    return bass.AP(tensor=t.tensor, offset=t.offset + off_elems,
                   ap=[t.ap[0], d1, d2])


def _build_bass(compile=True):
    nc = bacc.Bacc("TRN2", target_bir_lowering=False, debug=False,
                   num_devices=NCORES)
    x = nc.declare_dram_parameter("x", [TOTAL], mybir.dt.int8, isOutput=False)
    # sc packs sx | so | corr as one [P, 2*HPC*NT + 1] f32 block (one DMA)
    NCOL = HPC * NT
    sc = nc.declare_dram_parameter("sc", [P, 2 * NCOL + 1], mybir.dt.float32,
                                   isOutput=False)
    y = nc.declare_dram_parameter("y", [TOTAL], mybir.dt.int8, isOutput=True)

    with ExitStack() as ctx:
        tc = ctx.enter_context(tile.TileContext(nc))
        singles = ctx.enter_context(tc.tile_pool(name="singles", bufs=1))
        xpool = ctx.enter_context(tc.tile_pool(name="xp", bufs=7))
        epool = ctx.enter_context(tc.tile_pool(name="ep", bufs=5))
        spool = ctx.enter_context(tc.tile_pool(name="sp", bufs=4))
        ypool = ctx.enter_context(tc.tile_pool(name="yp", bufs=7))
        smalls = ctx.enter_context(tc.tile_pool(name="sm", bufs=8))

        # dummy activation on scratch SBUF: walrus emits the exp table load
        # right before it, so the ~1.3us ACT_TABLE_LOAD overlaps the first
        # DMA wait instead of serializing before the first real exp
        dummy = singles.tile([P, 1], mybir.dt.float32)
        nc.scalar.activation(dummy[:], dummy[:], AF.Exp, bias=0.0, scale=0.0)

        sct = singles.tile([P, 2 * NCOL + 1], mybir.dt.float32)
        nc.sync.dma_start(sct[:], sc[:])
        corrt = sct[:, 2 * NCOL:2 * NCOL + 1]

        def emit_exp_and_sums(h, t, sums, base):
            """DMA x in, exp -> et, row sums -> sums[:, base:base+B]."""
            off, W = _BLK[h][t]
            col = h * NT + t
            xt = xpool.tile([P, B * W], mybir.dt.int8, tag="xt")
            nc.sync.dma_start(
                xt[:], x[off:off + P * B * W].rearrange("(p n) -> p n", p=P))
            et = epool.tile([P, B * W], mybir.dt.float16, tag="et")
            if t < ACT_SUM_T0:
                # batched exp; row sums on DVE: halve in 2x fp16 adds,
                # then one 1x tensor_reduce over the shrunk block
                nc.scalar.activation(et[:], xt[:], AF.Exp, bias=0.0,
                                     scale=sct[:, col:col + 1])
                cur, w = et, W
                for lev in range(LEVELS[t]):
                    w2 = w // 2
                    # fixed max-size scratch per level (pool tags want
                    # stable shapes); APs below use only B*w2 elements
                    scr = spool.tile([P, 1280 >> lev],
                                     mybir.dt.float16, tag=f"scr{lev}")
                    nc.vector.tensor_tensor(
                        _ap3(scr, 0, P, [w2, B], [1, w2]),
                        _ap3(cur, 0, P, [w, B], [1, w2]),
                        _ap3(cur, w2, P, [w, B], [1, w2]),
                        ALU.add)
                    cur, w = scr, w2
                nc.vector.tensor_reduce(
                    sums[:, base:base + B], _ap3(cur, 0, P, [w, B], [1, w]),
                    AX.X, ALU.add)
            else:
                # per-b exp with free row sums from the ACT accumulator
                for b in range(B):
                    nc.scalar.activation(et[:, b * W:(b + 1) * W],
                                         xt[:, b * W:(b + 1) * W],
                                         AF.Exp, bias=0.0,
                                         scale=sct[:, col:col + 1],
                                         accum_out=sums[:, base + b:base + b + 1])
            return et

        def emit_requant(h, t, et, rt, base):
            """yt = et * rt[:, base+b] -> int8, DMA out."""
            off, W = _BLK[h][t]
            yt = ypool.tile([P, B * W], mybir.dt.int8, tag="yt")
            for b in range(B):
                nc.vector.tensor_scalar(yt[:, b * W:(b + 1) * W],
                                        et[:, b * W:(b + 1) * W],
                                        rt[:, base + b:base + b + 1], None,
                                        ALU.mult)
            nc.sync.dma_start(
                y[off:off + P * B * W].rearrange("(p n) -> p n", p=P), yt[:])

        # ascending order keeps ACT gapless (small DMAs land first, engines
        # self-balance); h1 rotates so the final tile is t0, whose short
        # post-exp chain (reduce+smalls+requant+64KB DMA) minimizes the tail
        TORD0 = [0, 1, 2, 3, 4, 5, 6, 7]
        TORD1 = [1, 2, 3, 4, 6, 7, 5, 0]
        for h in range(HPC):
            for t in (TORD0 if h == 0 else TORD1):
                col = h * NT + t
                sums = smalls.tile([P, B], mybir.dt.float32, tag="sums")
                et = emit_exp_and_sums(h, t, sums, 0)
                # r = 1/((sum - corr) * so): one fused op + fast reciprocal
                rt = smalls.tile([P, B], mybir.dt.float32, tag="rt")
                so_b = bass.AP(tensor=sct.tensor,
                               offset=sct.offset + NCOL + col,
                               ap=[sct.ap[0], [0, B]])
                nc.vector.scalar_tensor_tensor(rt[:], sums[:], corrt, so_b,
                                               ALU.subtract, ALU.mult)
                nc.vector.reciprocal_approx_fast(rt[:], rt[:])
                emit_requant(h, t, et, rt, 0)
    if compile:
        nc.compile()
    return nc


_tril_mask = None
_tril_small = None


def _host_prep(x_q, scale_x, scale_out):
    global _tril_mask
    x_q = np.asarray(x_q)
    assert x_q.dtype == np.int8, x_q.dtype
    scale_x = np.asarray(scale_x, dtype=np.float32).reshape(H, S)
    scale_out = np.asarray(scale_out, dtype=np.float32).reshape(H, S)

    if _tril_mask is None:
        _tril_mask = np.tril(np.ones((S, S), dtype=np.int8))
    x_pm = x_q * _tril_mask  # zero the strict upper triangle

    # [P, H, NT]: sxr[p, h, t] = scale_x[h, t*128 + p]
    sxr = scale_x.reshape(H, NT, P).transpose(2, 0, 1)
    sor = scale_out.reshape(H, NT, P).transpose(2, 0, 1)

    corr = (127 - np.arange(P)).astype(np.float32).reshape(P, 1)

    NCOL = HPC * NT
    in_maps = []
    for c in range(NCORES):
        xc = np.empty(TOTAL, np.int8)
        for h in range(HPC):
            hg = c * HPC + h
            for t in range(NT):
                off, W = _BLK[h][t]
                # [B, P, W] -> [P, B, W] flattened
                blk = x_pm[:, hg, t * P:(t + 1) * P, 0:W].transpose(1, 0, 2)
                xc[off:off + P * B * W] = blk.reshape(-1)
        hs = slice(c * HPC, (c + 1) * HPC)
        scc = np.empty((P, 2 * NCOL + 1), np.float32)
        scc[:, 0:NCOL] = sxr[:, hs].reshape(P, NCOL)
        scc[:, NCOL:2 * NCOL] = sor[:, hs].reshape(P, NCOL)
        scc[:, 2 * NCOL:] = corr
        in_maps.append({"x": xc, "sc": scc})
    return in_maps


def _host_unpack(results):
    global _tril_small
    if _tril_small is None:
        _tril_small = np.tril(np.ones((P, P), dtype=np.int8))
    out = np.zeros((B, H, S, S), np.int8)
    for c in range(NCORES):
        yc = np.asarray(results[c]["y"])
        for h in range(HPC):
            hg = c * HPC + h
            for t in range(NT):
                off, W = _BLK[h][t]
                blk = yc[off:off + P * B * W].reshape(P, B, W).transpose(1, 0, 2)
                out[:, hg, t * P:(t + 1) * P, 0:W] = blk
                # mask the strict upper triangle of the diagonal block
                out[:, hg, t * P:(t + 1) * P, t * P:(t + 1) * P] *= _tril_small[None]
    return out


def run(x_q, scale_x, scale_out, trace=False):
    global _cached_nc
    if trace:
        _ensure_ntff_hook()
    if _cached_nc is None:
        _cached_nc = _build_bass()
    in_maps = _host_prep(x_q, scale_x, scale_out)
    res = run_bass_kernel_spmd(_cached_nc, in_maps, core_ids=list(range(NCORES)),
                               trace=trace)
    return _host_unpack(res.results), res


def kernel(x_q, scale_x, scale_out):
    out, _ = run(x_q, scale_x, scale_out,
                 trace=bool(int(os.environ.get("KERNEL_TRACE", "0"))))
    return out


# revision 20
# speedup vs baseline: 1.0214x; 1.0090x over previous
"""Fused int8 dequant -> causal mask -> softmax -> int8 requant on 8 TRN2 cores.

Problem: x_q [B=4, H=16, S=1024, S] int8, per-(head,row) scales sx/so [H*S] f32.
  out = int8(clip(round(softmax(causal_mask(x_q * sx)) / so), -128, 127))

Sharding: 2 heads per core (data parallel over 64 independent (b, h) planes).
Rows live on partitions; softmax runs along the free dim. For each (h, t)
row-tile of 128 rows, only cols [0, W=(t+1)*128) can be nonzero (causal), so
only those are moved. Host-side prep packs x premasked (strict upper triangle
zeroed) into per-(h,t) blocks [128, B*W]; host-side unpack re-applies the
tril mask on the diagonal 128x128 block of each row-tile (so the device never
spends time zeroing masked lanes).

Engine budget (measured rates: ACT instr 369ns + 0.833ns/elem/lane, accum
readout 280ns; DVE fp16 tensor_scalar 2x = 0.52ns/elem, tensor_tensor 2x,
reductions 1x = 1.04ns/elem, small-op ~150-220ns/instr):

  ACT: one batched Exp per (h, t<=4) block [P, B*W] (row sums for these
       tiles are cheaper on DVE), per-b Exp+accum_out for t>=5 (large tiles,
       where DVE's 1x reduce tax exceeds ACT's per-instr+readout tax).
  DVE: row sums for t<=4 via a log-tree of fp16 tensor_tensor halving adds
       (2x mode) followed by one 1x tensor_reduce on the shrunk tile;
       r = 1/((sum - corr)*so) via one fused scalar_tensor_tensor +
       reciprocal; requant y = et * r -> int8 per (t,b) (2x mode).
  Premasked x makes masked lanes contribute exp(0)=1, corrected by the
  compile-time constant (127 - p) before use.

(fp16 et: element rounding gives end-to-end absmax diff 1 vs the f32
reference; sums accumulate in f32, halving partials stay in fp16.)
"""

import contextlib
import ctypes
import os
import sys
import types
from contextlib import ExitStack

import numpy as np

import concourse.bacc as bacc
import concourse.bass as bass
import concourse.tile as tile
from concourse import mybir
from concourse.bass_utils import run_bass_kernel_spmd

B, H, S = 4, 16, 1024
NCORES = 8
HPC = H // NCORES  # heads per core
P = 128
NT = S // P  # row tiles per plane
AF = mybir.ActivationFunctionType
ALU = mybir.AluOpType
AX = mybir.AxisListType

# packed block offsets: block (h, t) holds [P, B*W] int8, W = (t+1)*P
_BLK = [[None] * NT for _ in range(HPC)]
_off = 0
for _h in range(HPC):
    for _t in range(NT):
        _W = (_t + 1) * P
        _BLK[_h][_t] = (_off, _W)
        _off += P * B * _W
TOTAL = _off  # per-core packed bytes (4718592)

# tiles t < ACT_SUM_T0 sum on DVE (halving tree + tensor_reduce); the rest
# use per-b exp+accum on ACT. LEVELS[t] = halving-tree depth for DVE tiles.
ACT_SUM_T0 = 5
LEVELS = {0: 0, 1: 1, 2: 2, 3: 2, 4: 3}

_AXON_SO = "/opt/axon/libaxon_pjrt.so"


def _ensure_ntff_hook():
    """This image's antenv lacks axon_hooks; provide it so trace=True works."""
    if "antenv.axon_hooks" in sys.modules:
        return
    import antenv

    mod = types.ModuleType("antenv.axon_hooks")
    state = {"hook": None}
    mod.set_axon_ntff_profile_hook = lambda h: state.__setitem__("hook", h)
    mod.get_axon_ntff_profile_hook = lambda: state["hook"]
    sys.modules["antenv.axon_hooks"] = mod
    antenv.axon_hooks = mod

    if not os.path.exists(_AXON_SO):
        return
    lib = ctypes.CDLL(_AXON_SO)
    if not hasattr(lib, "axon_start_nrt_profile"):
        return
    lib.axon_start_nrt_profile.argtypes = [ctypes.POINTER(ctypes.c_int64), ctypes.c_size_t]
    lib.axon_start_nrt_profile.restype = ctypes.c_int64
    lib.axon_stop_nrt_profile.argtypes = [ctypes.c_char_p]
    lib.axon_stop_nrt_profile.restype = ctypes.c_int64

    @contextlib.contextmanager
    def _hook(output_dir, device_ids):
        import jax

        jax.devices()
        if device_ids:
            ids = (ctypes.c_int64 * len(device_ids))(*device_ids)
            rc = lib.axon_start_nrt_profile(ids, len(device_ids))
        else:
            rc = lib.axon_start_nrt_profile(None, 0)
        if rc != 0:
            raise RuntimeError(f"axon_start_nrt_profile rc={rc}")
        try:
            yield
        finally:
            n = lib.axon_stop_nrt_profile(str(output_dir).encode())
            print(f"profile: {n} file(s) written to {output_dir}", file=sys.stderr)

    mod.set_axon_ntff_profile_hook(_hook)


_cached_nc = None


def _ap3(t, off_elems, pdim, d1, d2):
    """3D AP view [partitions, d1, d2] of tile t at element offset off_elems."""
    return bass.AP(tensor=t.tensor, offset=t.offset + off_elems,
                   ap=[t.ap[0], d1, d2])


def _build_bass(compile=True):
    nc = bacc.Bacc("TRN2", target_bir_lowering=False, debug=False,
                   num_devices=NCORES)
    x = nc.declare_dram_parameter("x", [TOTAL], mybir.dt.int8, isOutput=False)
    # sc packs sx | so | corr as one [P, 2*HPC*NT + 1] f32 block (one DMA)
    NCOL = HPC * NT
    sc = nc.declare_dram_parameter("sc", [P, 2 * NCOL + 1], mybir.dt.float32,
                                   isOutput=False)
    y = nc.declare_dram_parameter("y", [TOTAL], mybir.dt.int8, isOutput=True)

    with ExitStack() as ctx:
        tc = ctx.enter_context(tile.TileContext(nc))
        singles = ctx.enter_context(tc.tile_pool(name="singles", bufs=1))
        xpool = ctx.enter_context(tc.tile_pool(name="xp", bufs=7))
        epool = ctx.enter_context(tc.tile_pool(name="ep", bufs=5))
        spool = ctx.enter_context(tc.tile_pool(name="sp", bufs=4))
        ypool = ctx.enter_context(tc.tile_pool(name="yp", bufs=7))
        smalls = ctx.enter_context(tc.tile_pool(name="sm", bufs=8))

        # dummy activation on scratch SBUF: walrus emits the exp table load
        # right before it, so the ~1.3us ACT_TABLE_LOAD overlaps the first
        # DMA wait instead of serializing before the first real exp
        dummy = singles.tile([P, 1], mybir.dt.float32)
        nc.scalar.activation(dummy[:], dummy[:], AF.Exp, bias=0.0, scale=0.0)

        sct = singles.tile([P, 2 * NCOL + 1], mybir.dt.float32)
        nc.sync.dma_start(sct[:], sc[:])
        corrt = sct[:, 2 * NCOL:2 * NCOL + 1]

        def emit_exp_and_sums(h, t, sums, base, force_act=False):
            """DMA x in, exp -> et, row sums -> sums[:, base:base+B]."""
            off, W = _BLK[h][t]
            col = h * NT + t
            xt = xpool.tile([P, B * W], mybir.dt.int8, tag="xt")
            nc.sync.dma_start(
                xt[:], x[off:off + P * B * W].rearrange("(p n) -> p n", p=P))
            et = epool.tile([P, B * W], mybir.dt.float16, tag="et")
            if t < ACT_SUM_T0 and not force_act:
                # batched exp; row sums on DVE: halve in 2x fp16 adds,
                # then one 1x tensor_reduce over the shrunk block
                nc.scalar.activation(et[:], xt[:], AF.Exp, bias=0.0,
                                     scale=sct[:, col:col + 1])
                cur, w = et, W
                for lev in range(LEVELS[t]):
                    w2 = w // 2
                    # fixed max-size scratch per level (pool tags want
                    # stable shapes); APs below use only B*w2 elements
                    scr = spool.tile([P, 1280 >> lev],
                                     mybir.dt.float16, tag=f"scr{lev}")
                    nc.vector.tensor_tensor(
                        _ap3(scr, 0, P, [w2, B], [1, w2]),
                        _ap3(cur, 0, P, [w, B], [1, w2]),
                        _ap3(cur, w2, P, [w, B], [1, w2]),
                        ALU.add)
                    cur, w = scr, w2
                nc.vector.tensor_reduce(
                    sums[:, base:base + B], _ap3(cur, 0, P, [w, B], [1, w]),
                    AX.X, ALU.add)
            else:
                # per-b exp with free row sums from the ACT accumulator
                for b in range(B):
                    nc.scalar.activation(et[:, b * W:(b + 1) * W],
                                         xt[:, b * W:(b + 1) * W],
                                         AF.Exp, bias=0.0,
                                         scale=sct[:, col:col + 1],
                                         accum_out=sums[:, base + b:base + b + 1])
            return et

        def emit_requant(h, t, et, rt, base):
            """yt = et * rt[:, base+b] -> int8, DMA out."""
            off, W = _BLK[h][t]
            yt = ypool.tile([P, B * W], mybir.dt.int8, tag="yt")
            for b in range(B):
                nc.vector.tensor_scalar(yt[:, b * W:(b + 1) * W],
                                        et[:, b * W:(b + 1) * W],
                                        rt[:, base + b:base + b + 1], None,
                                        ALU.mult)
            nc.sync.dma_start(
                y[off:off + P * B * W].rearrange("(p n) -> p n", p=P), yt[:])

        # ascending order keeps ACT gapless (small DMAs land first, engines
        # self-balance); h1 rotates so the final tile is t0, whose short
        # post-exp chain (reduce+smalls+requant+64KB DMA) minimizes the tail
        TORD0 = [0, 1, 2, 3, 4, 5, 6, 7]
        TORD1 = [1, 2, 3, 4, 6, 7, 5, 0]
        for h in range(HPC):
            for t in (TORD0 if h == 0 else TORD1):
                col = h * NT + t
                sums = smalls.tile([P, B], mybir.dt.float32, tag="sums")
                # the very last tile (t0 of h1) uses ACT-accum sums: its
                # post-ACT DVE chain drops the tensor_reduce and overlaps
                # the preceding tile's chain, shortening the kernel tail
                last = (h == HPC - 1) and (t == 0)
                et = emit_exp_and_sums(h, t, sums, 0, force_act=last)
                # r = 1/((sum - corr) * so): one fused op + fast reciprocal
                rt = smalls.tile([P, B], mybir.dt.float32, tag="rt")
                so_b = bass.AP(tensor=sct.tensor,
                               offset=sct.offset + NCOL + col,
                               ap=[sct.ap[0], [0, B]])
                nc.vector.scalar_tensor_tensor(rt[:], sums[:], corrt, so_b,
                                               ALU.subtract, ALU.mult)
                nc.vector.reciprocal_approx_fast(rt[:], rt[:])
                emit_requant(h, t, et, rt, 0)
    if compile:
        nc.compile()
    return nc


_tril_mask = None
_tril_small = None


def _host_prep(x_q, scale_x, scale_out):
    global _tril_mask
    x_q = np.asarray(x_q)
    assert x_q.dtype == np.int8, x_q.dtype
    scale_x = np.asarray(scale_x, dtype=np.float32).reshape(H, S)
    scale_out = np.asarray(scale_out, dtype=np.float32).reshape(H, S)

    if _tril_mask is None:
        _tril_mask = np.tril(np.ones((S, S), dtype=np.int8))
    x_pm = x_q * _tril_mask  # zero the strict upper triangle

    # [P, H, NT]: sxr[p, h, t] = scale_x[h, t*128 + p]
    sxr = scale_x.reshape(H, NT, P).transpose(2, 0, 1)
    sor = scale_out.reshape(H, NT, P).transpose(2, 0, 1)

    corr = (127 - np.arange(P)).astype(np.float32).reshape(P, 1)

    NCOL = HPC * NT
    in_maps = []
    for c in range(NCORES):
        xc = np.empty(TOTAL, np.int8)
        for h in range(HPC):
            hg = c * HPC + h
            for t in range(NT):
                off, W = _BLK[h][t]
                # [B, P, W] -> [P, B, W] flattened
                blk = x_pm[:, hg, t * P:(t + 1) * P, 0:W].transpose(1, 0, 2)
                xc[off:off + P * B * W] = blk.reshape(-1)
        hs = slice(c * HPC, (c + 1) * HPC)
        scc = np.empty((P, 2 * NCOL + 1), np.float32)
        scc[:, 0:NCOL] = sxr[:, hs].reshape(P, NCOL)
        scc[:, NCOL:2 * NCOL] = sor[:, hs].reshape(P, NCOL)
        scc[:, 2 * NCOL:] = corr
        in_maps.append({"x": xc, "sc": scc})
    return in_maps


def _host_unpack(results):
    global _tril_small
    if _tril_small is None:
        _tril_small = np.tril(np.ones((P, P), dtype=np.int8))
    out = np.zeros((B, H, S, S), np.int8)
    for c in range(NCORES):
        yc = np.asarray(results[c]["y"])
        for h in range(HPC):
            hg = c * HPC + h
            for t in range(NT):
                off, W = _BLK[h][t]
                blk = yc[off:off + P * B * W].reshape(P, B, W).transpose(1, 0, 2)
                out[:, hg, t * P:(t + 1) * P, 0:W] = blk
                # mask the strict upper triangle of the diagonal block
                out[:, hg, t * P:(t + 1) * P, t * P:(t + 1) * P] *= _tril_small[None]
    return out


def run(x_q, scale_x, scale_out, trace=False):
    global _cached_nc
    if trace:
        _ensure_ntff_hook()
    if _cached_nc is None:
        _cached_nc = _build_bass()
    in_maps = _host_prep(x_q, scale_x, scale_out)
    res = run_bass_kernel_spmd(_cached_nc, in_maps, core_ids=list(range(NCORES)),
                               trace=trace)
    return _host_unpack(res.results), res


def kernel(x_q, scale_x, scale_out):
    out, _ = run(x_q, scale_x, scale_out,
                 trace=bool(int(os.environ.get("KERNEL_TRACE", "0"))))
    return out


# revision 21
# speedup vs baseline: 1.0260x; 1.0045x over previous
"""Fused int8 dequant -> causal mask -> softmax -> int8 requant on 8 TRN2 cores.

Problem: x_q [B=4, H=16, S=1024, S] int8, per-(head,row) scales sx/so [H*S] f32.
  out = int8(clip(round(softmax(causal_mask(x_q * sx)) / so), -128, 127))

Sharding: 2 heads per core (data parallel over 64 independent (b, h) planes).
Rows live on partitions; softmax runs along the free dim. For each (h, t)
row-tile of 128 rows, only cols [0, W=(t+1)*128) can be nonzero (causal), so
only those are moved. Host-side prep packs x premasked (strict upper triangle
zeroed) into per-(h,t) blocks [128, B*W]; host-side unpack re-applies the
tril mask on the diagonal 128x128 block of each row-tile (so the device never
spends time zeroing masked lanes).

Engine budget (measured rates: ACT instr 369ns + 0.833ns/elem/lane, accum
readout 280ns; DVE fp16 tensor_scalar 2x = 0.52ns/elem, tensor_tensor 2x,
reductions 1x = 1.04ns/elem, small-op ~150-220ns/instr):

  ACT: one batched Exp per (h, t<=4) block [P, B*W] (row sums for these
       tiles are cheaper on DVE), per-b Exp+accum_out for t>=5 (large tiles,
       where DVE's 1x reduce tax exceeds ACT's per-instr+readout tax).
  DVE: row sums for t<=4 via a log-tree of fp16 tensor_tensor halving adds
       (2x mode) followed by one 1x tensor_reduce on the shrunk tile;
       r = 1/((sum - corr)*so) via one fused scalar_tensor_tensor +
       reciprocal_approx_fast; requant y = et * r -> int8 per (t,b)
       (2x mode). Premasked x makes masked lanes contribute exp(0)=1,
       corrected by the compile-time constant (127 - p) before use.

Pipeline shape (measured): ~7us fixed NEFF preamble, then a dummy Exp pulls
the ~1.3us ACT table load under the first DMA wait; sx|so|corr ship as one
merged DMA. Ascending tile order keeps ACT gapless; h1 is rotated so the
kernel ends on t0, which also uses ACT-accum sums so the final DVE chain is
just smalls+requant (short tail). Engines balance at ~46-48us busy each;
TensorE can't help (matmul only contracts the partition dim and PSUM exit
costs a 1x pass) and GPSIMD compute would deadlock-serialize against the
requant's 2-port DVE mode (shared SBUF port-pair lock).

(fp16 et: element rounding gives end-to-end absmax diff 1 vs the f32
reference; sums accumulate in f32, halving partials stay in fp16.)
"""

import contextlib
import ctypes
import os
import sys
import types
from contextlib import ExitStack

import numpy as np

import concourse.bacc as bacc
import concourse.bass as bass
import concourse.tile as tile
from concourse import mybir
from concourse.bass_utils import run_bass_kernel_spmd

B, H, S = 4, 16, 1024
NCORES = 8
HPC = H // NCORES  # heads per core
P = 128
NT = S // P  # row tiles per plane
AF = mybir.ActivationFunctionType
ALU = mybir.AluOpType
AX = mybir.AxisListType

# packed block offsets: block (h, t) holds [P, B*W] int8, W = (t+1)*P
_BLK = [[None] * NT for _ in range(HPC)]
_off = 0
for _h in range(HPC):
    for _t in range(NT):
        _W = (_t + 1) * P
        _BLK[_h][_t] = (_off, _W)
        _off += P * B * _W
TOTAL = _off  # per-core packed bytes (4718592)

# tiles t < ACT_SUM_T0 sum on DVE (halving tree + tensor_reduce); the rest
# use per-b exp+accum on ACT. LEVELS[t] = halving-tree depth for DVE tiles.
ACT_SUM_T0 = 5
LEVELS = {0: 0, 1: 1, 2: 2, 3: 2, 4: 3}

_AXON_SO = "/opt/axon/libaxon_pjrt.so"


def _ensure_ntff_hook():
    """This image's antenv lacks axon_hooks; provide it so trace=True works."""
    if "antenv.axon_hooks" in sys.modules:
        return
    import antenv

    mod = types.ModuleType("antenv.axon_hooks")
    state = {"hook": None}
    mod.set_axon_ntff_profile_hook = lambda h: state.__setitem__("hook", h)
    mod.get_axon_ntff_profile_hook = lambda: state["hook"]
    sys.modules["antenv.axon_hooks"] = mod
    antenv.axon_hooks = mod

    if not os.path.exists(_AXON_SO):
        return
    lib = ctypes.CDLL(_AXON_SO)
    if not hasattr(lib, "axon_start_nrt_profile"):
        return
    lib.axon_start_nrt_profile.argtypes = [ctypes.POINTER(ctypes.c_int64), ctypes.c_size_t]
    lib.axon_start_nrt_profile.restype = ctypes.c_int64
    lib.axon_stop_nrt_profile.argtypes = [ctypes.c_char_p]
    lib.axon_stop_nrt_profile.restype = ctypes.c_int64

    @contextlib.contextmanager
    def _hook(output_dir, device_ids):
        import jax

        jax.devices()
        if device_ids:
            ids = (ctypes.c_int64 * len(device_ids))(*device_ids)
            rc = lib.axon_start_nrt_profile(ids, len(device_ids))
        else:
            rc = lib.axon_start_nrt_profile(None, 0)
        if rc != 0:
            raise RuntimeError(f"axon_start_nrt_profile rc={rc}")
        try:
            yield
        finally:
            n = lib.axon_stop_nrt_profile(str(output_dir).encode())
            print(f"profile: {n} file(s) written to {output_dir}", file=sys.stderr)

    mod.set_axon_ntff_profile_hook(_hook)


_cached_nc = None


def _ap3(t, off_elems, pdim, d1, d2):
    """3D AP view [partitions, d1, d2] of tile t at element offset off_elems."""
    return bass.AP(tensor=t.tensor, offset=t.offset + off_elems,
                   ap=[t.ap[0], d1, d2])


def _build_bass(compile=True):
    nc = bacc.Bacc("TRN2", target_bir_lowering=False, debug=False,
                   num_devices=NCORES)
    x = nc.declare_dram_parameter("x", [TOTAL], mybir.dt.int8, isOutput=False)
    # sc packs sx | so | corr as one [P, 2*HPC*NT + 1] f32 block (one DMA)
    NCOL = HPC * NT
    sc = nc.declare_dram_parameter("sc", [P, 2 * NCOL + 1], mybir.dt.float32,
                                   isOutput=False)
    y = nc.declare_dram_parameter("y", [TOTAL], mybir.dt.int8, isOutput=True)

    with ExitStack() as ctx:
        tc = ctx.enter_context(tile.TileContext(nc))
        singles = ctx.enter_context(tc.tile_pool(name="singles", bufs=1))
        xpool = ctx.enter_context(tc.tile_pool(name="xp", bufs=7))
        epool = ctx.enter_context(tc.tile_pool(name="ep", bufs=5))
        spool = ctx.enter_context(tc.tile_pool(name="sp", bufs=4))
        ypool = ctx.enter_context(tc.tile_pool(name="yp", bufs=7))
        smalls = ctx.enter_context(tc.tile_pool(name="sm", bufs=8))

        # dummy activation on scratch SBUF: walrus emits the exp table load
        # right before it, so the ~1.3us ACT_TABLE_LOAD overlaps the first
        # DMA wait instead of serializing before the first real exp
        dummy = singles.tile([P, 1], mybir.dt.float32)
        nc.scalar.activation(dummy[:], dummy[:], AF.Exp, bias=0.0, scale=0.0)

        sct = singles.tile([P, 2 * NCOL + 1], mybir.dt.float32)
        nc.sync.dma_start(sct[:], sc[:])
        corrt = sct[:, 2 * NCOL:2 * NCOL + 1]

        def emit_exp_and_sums(h, t, sums, base, force_act=False):
            """DMA x in, exp -> et, row sums -> sums[:, base:base+B]."""
            off, W = _BLK[h][t]
            col = h * NT + t
            xt = xpool.tile([P, B * W], mybir.dt.int8, tag="xt")
            nc.sync.dma_start(
                xt[:], x[off:off + P * B * W].rearrange("(p n) -> p n", p=P))
            et = epool.tile([P, B * W], mybir.dt.float16, tag="et")
            if t < ACT_SUM_T0 and not force_act:
                # batched exp; row sums on DVE: halve in 2x fp16 adds,
                # then one 1x tensor_reduce over the shrunk block
                nc.scalar.activation(et[:], xt[:], AF.Exp, bias=0.0,
                                     scale=sct[:, col:col + 1])
                cur, w = et, W
                for lev in range(LEVELS[t]):
                    w2 = w // 2
                    # fixed max-size scratch per level (pool tags want
                    # stable shapes); APs below use only B*w2 elements
                    scr = spool.tile([P, 1280 >> lev],
                                     mybir.dt.float16, tag=f"scr{lev}")
                    nc.vector.tensor_tensor(
                        _ap3(scr, 0, P, [w2, B], [1, w2]),
                        _ap3(cur, 0, P, [w, B], [1, w2]),
                        _ap3(cur, w2, P, [w, B], [1, w2]),
                        ALU.add)
                    cur, w = scr, w2
                nc.vector.tensor_reduce(
                    sums[:, base:base + B], _ap3(cur, 0, P, [w, B], [1, w]),
                    AX.X, ALU.add)
            else:
                # per-b exp with free row sums from the ACT accumulator
                for b in range(B):
                    nc.scalar.activation(et[:, b * W:(b + 1) * W],
                                         xt[:, b * W:(b + 1) * W],
                                         AF.Exp, bias=0.0,
                                         scale=sct[:, col:col + 1],
                                         accum_out=sums[:, base + b:base + b + 1])
            return et

        def emit_requant(h, t, et, rt, base):
            """yt = et * rt[:, base+b] -> int8, DMA out."""
            off, W = _BLK[h][t]
            yt = ypool.tile([P, B * W], mybir.dt.int8, tag="yt")
            for b in range(B):
                nc.vector.tensor_scalar(yt[:, b * W:(b + 1) * W],
                                        et[:, b * W:(b + 1) * W],
                                        rt[:, base + b:base + b + 1], None,
                                        ALU.mult)
            nc.sync.dma_start(
                y[off:off + P * B * W].rearrange("(p n) -> p n", p=P), yt[:])

        # ascending order keeps ACT gapless (small DMAs land first, engines
        # self-balance); h1 rotates so the final tile is t0, whose short
        # post-exp chain (reduce+smalls+requant+64KB DMA) minimizes the tail
        TORD0 = [0, 1, 2, 3, 4, 5, 6, 7]
        TORD1 = [1, 2, 3, 4, 6, 7, 5, 0]
        for h in range(HPC):
            for t in (TORD0 if h == 0 else TORD1):
                col = h * NT + t
                sums = smalls.tile([P, B], mybir.dt.float32, tag="sums")
                # the very last tile (t0 of h1) uses ACT-accum sums: its
                # post-ACT DVE chain drops the tensor_reduce and overlaps
                # the preceding tile's chain, shortening the kernel tail
                last = (h == HPC - 1) and (t == 0)
                et = emit_exp_and_sums(h, t, sums, 0, force_act=last)
                # r = 1/((sum - corr) * so): one fused op + fast reciprocal
                rt = smalls.tile([P, B], mybir.dt.float32, tag="rt")
                so_b = bass.AP(tensor=sct.tensor,
                               offset=sct.offset + NCOL + col,
                               ap=[sct.ap[0], [0, B]])
                nc.vector.scalar_tensor_tensor(rt[:], sums[:], corrt, so_b,
                                               ALU.subtract, ALU.mult)
                nc.vector.reciprocal_approx_fast(rt[:], rt[:])
                emit_requant(h, t, et, rt, 0)
    if compile:
        nc.compile()
    return nc


_tril_mask = None
_tril_small = None


def _host_prep(x_q, scale_x, scale_out):
    global _tril_mask
    x_q = np.asarray(x_q)
    assert x_q.dtype == np.int8, x_q.dtype
    scale_x = np.asarray(scale_x, dtype=np.float32).reshape(H, S)
    scale_out = np.asarray(scale_out, dtype=np.float32).reshape(H, S)

    if _tril_mask is None:
        _tril_mask = np.tril(np.ones((S, S), dtype=np.int8))
    x_pm = x_q * _tril_mask  # zero the strict upper triangle

    # [P, H, NT]: sxr[p, h, t] = scale_x[h, t*128 + p]
    sxr = scale_x.reshape(H, NT, P).transpose(2, 0, 1)
    sor = scale_out.reshape(H, NT, P).transpose(2, 0, 1)

    corr = (127 - np.arange(P)).astype(np.float32).reshape(P, 1)

    NCOL = HPC * NT
    in_maps = []
    for c in range(NCORES):
        xc = np.empty(TOTAL, np.int8)
        for h in range(HPC):
            hg = c * HPC + h
            for t in range(NT):
                off, W = _BLK[h][t]
                # [B, P, W] -> [P, B, W] flattened
                blk = x_pm[:, hg, t * P:(t + 1) * P, 0:W].transpose(1, 0, 2)
                xc[off:off + P * B * W] = blk.reshape(-1)
        hs = slice(c * HPC, (c + 1) * HPC)
        scc = np.empty((P, 2 * NCOL + 1), np.float32)
        scc[:, 0:NCOL] = sxr[:, hs].reshape(P, NCOL)
        scc[:, NCOL:2 * NCOL] = sor[:, hs].reshape(P, NCOL)
        scc[:, 2 * NCOL:] = corr
        in_maps.append({"x": xc, "sc": scc})
    return in_maps


def _host_unpack(results):
    global _tril_small
    if _tril_small is None:
        _tril_small = np.tril(np.ones((P, P), dtype=np.int8))
    out = np.zeros((B, H, S, S), np.int8)
    for c in range(NCORES):
        yc = np.asarray(results[c]["y"])
        for h in range(HPC):
            hg = c * HPC + h
            for t in range(NT):
                off, W = _BLK[h][t]
                blk = yc[off:off + P * B * W].reshape(P, B, W).transpose(1, 0, 2)
                out[:, hg, t * P:(t + 1) * P, 0:W] = blk
                # mask the strict upper triangle of the diagonal block
                out[:, hg, t * P:(t + 1) * P, t * P:(t + 1) * P] *= _tril_small[None]
    return out


def run(x_q, scale_x, scale_out, trace=False):
    global _cached_nc
    if trace:
        _ensure_ntff_hook()
    if _cached_nc is None:
        _cached_nc = _build_bass()
    in_maps = _host_prep(x_q, scale_x, scale_out)
    res = run_bass_kernel_spmd(_cached_nc, in_maps, core_ids=list(range(NCORES)),
                               trace=trace)
    return _host_unpack(res.results), res


def kernel(x_q, scale_x, scale_out):
    out, _ = run(x_q, scale_x, scale_out,
                 trace=bool(int(os.environ.get("KERNEL_TRACE", "0"))))
    return out
